# revision 35
# baseline (speedup 1.0000x reference)
"""BatchAllTripletLoss Trainium2 kernel (final).

Problem (hardcoded): x (64, 256, 256) f32, y (64, 256) int64 with
y[p, i] = i // 8 (32 classes x 8 members, uniform, identical across parts).
Output: per-part batch-all triplet loss, shape (64,) f32.

Design (8 cores x 8 parts, fully independent):
  - constants (fold lhsT/rhs templates, identity) DMA'd from a host-
    built DRAM blob on the ACT queue (no Pool affine_selects at boot)
  - x loaded f32 (sync DMA), cast to bf16 on GpSimd (Pool)
  - squared norms on ACT (Square + accum); x^T via PE transposes with
    the sqrow strip sharing the transpose PSUM bank
  - one PSUM accumulation per part [128,512]: gram (2 matmuls/half) +
    stacked [33,*] fold (row0 = -sqrow/2, rows 1-32 = -L/2 * CT)
  - D' = sqrt in bf16 on ACT; pos distances from the SAME psum:
    ps-space min over the anchor-half's 16-class block, clamped via
    sposc2 = min(2*spos, sq-L), pp = sqrt(-sposc2+(sq-L))
  - epilogue per (part, half): ALL 8 counts in ONE wide DVE
    scalar_tensor_tensor ((D - m) is_lt pp_broadcast, [128, 2048],
    in-op accumulated, t-major layout); relu-sums: M_WIDE slots in one
    wide DVE op using the exact min identity
    relu(pp+m-D) = pp - min(D-m, pp)  =>  S = N*sum(pp) - sum(min),
    the rest as narrow ACT Relu ops (bias=pm, scale=-1, accum)
  - per-core output: (S_p, N_p) pairs; host does the final division.
"""

import numpy as np
from contextlib import ExitStack

import concourse.bass as bass
import concourse.bacc as bacc_mod
import concourse.mybir as mybir
import concourse.tile as tile

F32 = mybir.dt.float32
BF16 = mybir.dt.bfloat16
ALU = mybir.AluOpType
ACTF = mybir.ActivationFunctionType

P_TOT, N, C = 64, 256, 256
K, NCLS = 8, 32
MARGIN = 0.2
NCORES = 8
PPC = P_TOT // NCORES
HALVES = 2
LBIG = float(2 << 19)
# slots 0..M_WIDE-1 of the relu-sums ride ONE wide DVE op (broadcast
# APs, min-form corrected by N*sum(pp)); the rest are narrow ACT Relu
# ops. All counts ride ONE wide DVE op per (part, half).
M_WIDE = 3
SA = K - M_WIDE          # narrow ACT relu-sum slots -> acc cols 0..SA-1
WSUM_COL = SA            # wide-sum accumulator col
PPSUM_COL = SA + 1       # sum of pp over wide slots (for min-form correction)
CNT_COL = SA + 2         # wide-count accumulator col
ACC_W = SA + 3


def build_kernel(do_compile=True, reps=1):
    nc = bacc_mod.Bacc()
    x_in = nc.declare_dram_parameter("x", [PPC * N, C], F32, isOutput=False)
    cst_in = nc.declare_dram_parameter("cst", [128, 640], BF16, isOutput=False)
    sn_out = nc.declare_dram_parameter("sn", [1, 2 * PPC], F32, isOutput=True)

    with tile.TileContext(nc) as tc, ExitStack() as ctx:
        consts = ctx.enter_context(tc.tile_pool(name="consts", bufs=1))
        xpool = ctx.enter_context(tc.tile_pool(name="xpool", bufs=2))
        xtpool = ctx.enter_context(tc.tile_pool(name="xtpool", bufs=2))
        dpool = ctx.enter_context(tc.tile_pool(name="dpool", bufs=4))
        small = ctx.enter_context(tc.tile_pool(name="small", bufs=6))
        trash = ctx.enter_context(tc.tile_pool(name="trash", bufs=8))
        accp = ctx.enter_context(tc.tile_pool(name="accp", bufs=3))
        psum = ctx.enter_context(tc.tile_pool(name="psum", bufs=2, space="PSUM"))
        psmall = ctx.enter_context(tc.tile_pool(name="psmall", bufs=1, space="PSUM"))

        # ---- one-time constants, DMA'd from DRAM on the ACT queue ----
        lhsT_st = consts.tile([NCLS + 1, N], BF16, tag="lhsT_st")
        nc.scalar.dma_start(lhsT_st[:], cst_in[0: NCLS + 1, 0:N])
        rhs_st = consts.tile([NCLS + 1, N], BF16, tag="rhs_st")
        nc.scalar.dma_start(rhs_st[:], cst_in[0: NCLS + 1, N: 2 * N])
        ident = consts.tile([128, 128], BF16, tag="ident")
        nc.scalar.dma_start(ident[:], cst_in[:, 2 * N: 2 * N + 128])
        ones_col = consts.tile([128, 1], F32, tag="ones_col")
        nc.gpsimd.memset(ones_col[:], 1.0)

        fin_ps = psmall.tile([1, ACC_W * HALVES * PPC], F32, tag="fin_ps")

        for p in [pp for _ in range(reps) for pp in range(PPC)]:
            # ---- load f32, cast to bf16 on Pool ----
            xf = [xpool.tile([128, C], F32, tag="xf", name="xf", bufs=6) for _ in range(HALVES)]
            for h in range(HALVES):
                nc.sync.dma_start(xf[h][:], x_in[p * N + 128 * h: p * N + 128 * (h + 1), :])
            xb = [xpool.tile([128, C], BF16, tag="xb", name="xb", bufs=4) for _ in range(HALVES)]
            for h in range(HALVES):
                nc.gpsimd.tensor_copy(xb[h][:], xf[h][:])

            # ---- squared norms ----
            sqcol2 = small.tile([128, HALVES], F32, tag="sqcol2")
            for h in range(HALVES):
                st = trash.tile([128, C], BF16, tag="trash_sq")
                nc.scalar.activation(
                    st[:], xb[h][:], ACTF.Square,
                    accum_out=sqcol2[:, h: h + 1],
                )
            scb = small.tile([128, HALVES], BF16, tag="scb")
            nc.gpsimd.tensor_copy(scb[:], sqcol2[:])

            # ---- transposes via PE ----
            xtps = psum.tile([128, 2 * N + 2 * 128], BF16, tag="xtps", name="xtps", bufs=2)
            for cchunk in range(2):
                for h in range(HALVES):
                    nc.tensor.transpose(
                        xtps[:, 256 * cchunk + 128 * h: 256 * cchunk + 128 * (h + 1)],
                        xb[h][:, 128 * cchunk: 128 * (cchunk + 1)],
                        ident[:],
                    )
            for h in range(HALVES):
                nc.tensor.transpose(
                    xtps[0:1, 2 * N + 128 * h: 2 * N + 128 * (h + 1)],
                    scb[:, h: h + 1], ident[:],
                )
            xtb_all = xtpool.tile([128, 2 * N], BF16, tag="xtb", name="xtb")
            nc.vector.tensor_copy(xtb_all[:], xtps[:, 0: 2 * N])
            xtb = [xtb_all[:, 0:N], xtb_all[:, N: 2 * N]]
            nc.vector.tensor_copy(rhs_st[0:1, :], xtps[0:1, 2 * N: 2 * N + 2 * 128])

            # ---- psum: gram + stacked fold ----
            ps = psum.tile([128, 2 * N], F32, tag="ps", bufs=3)
            for h in range(HALVES):
                psh = ps[:, N * h: N * (h + 1)]
                nc.tensor.matmul(
                    psh, xtb[0][:, 128 * h: 128 * (h + 1)], xtb[0][:],
                    start=True, stop=False,
                )
                nc.tensor.matmul(
                    psh, xtb[1][:, 128 * h: 128 * (h + 1)], xtb[1][:],
                    start=False, stop=False,
                )
                nc.tensor.matmul(
                    psh, lhsT_st[:, 128 * h: 128 * (h + 1)], rhs_st[:],
                    start=False, stop=True,
                )

            sqml = small.tile([128, HALVES], F32, tag="sqml")
            nc.vector.tensor_scalar(sqml[:], sqcol2[:], -LBIG, None, op0=ALU.add)

            acc = accp.tile([128, ACC_W * HALVES], F32, tag="acc", name="acc")
            dmat = []
            pm = []
            ppt = []
            for h in range(HALVES):
                psh = ps[:, N * h: N * (h + 1)]
                dm = dpool.tile([128, N], BF16, tag="dmat")
                nc.scalar.activation(
                    dm[:], psh, ACTF.Sqrt, bias=sqcol2[:, h: h + 1], scale=-2.0,
                )
                dmat.append(dm)

                spos = small.tile([128, K], F32, tag="spos")
                nc.vector.tensor_reduce(
                    spos[:],
                    ps[:, 384 * h: 384 * h + 128].rearrange(
                        "p (cc t) -> p t cc", cc=NCLS // 2, t=K
                    ),
                    axis=mybir.AxisListType.X, op=ALU.min,
                )
                sposc2 = small.tile([128, K], F32, tag="sposc2")
                nc.vector.tensor_scalar(
                    sposc2[:], spos[:], 2.0, sqml[:, h: h + 1],
                    op0=ALU.mult, op1=ALU.min,
                )
                pp = small.tile([128, K], F32, tag="pp")
                nc.scalar.activation(
                    pp[:], sposc2[:], ACTF.Sqrt, bias=sqml[:, h: h + 1], scale=-1.0,
                )
                ppt.append(pp)
                pmh = small.tile([128, K], F32, tag="pmh")
                nc.vector.tensor_scalar(pmh[:], pp[:], MARGIN, None, op0=ALU.add)
                pm.append(pmh)

            # ---- epilogue slots ----
            for h in range(HALVES):
                dm = dmat[h]
                base = ACC_W * h
                # wide count: all 8 slots in one broadcast-AP STT
                in0_w = dm[:, :].rearrange("p (o l) -> p o l", o=1).broadcast_to([128, K, N])
                in1_w = ppt[h][:, :].rearrange("p (t o) -> p t o", o=1).broadcast_to([128, K, N])
                tc_w = trash.tile([128, N * K], BF16, tag="trWC", bufs=4)
                nc.vector.scalar_tensor_tensor(
                    tc_w[:].rearrange("p (t l) -> p t l", t=K, l=N),
                    in0_w, -MARGIN, in1_w, op0=ALU.add, op1=ALU.is_lt,
                    accum_out=acc[:, base + CNT_COL: base + CNT_COL + 1],
                )
                # wide sum over slots 0..M_WIDE-1: accum = sum max(pm_t, D)
                # min form: relu(pp+m-D) = pp - min(D-m, pp), exactly; so
                # wide relu-sum = N*sum(pp[0:M]) - sum min(D-m, pp)
                in0_s = dm[:, :].rearrange("p (l o) -> p l o", o=1).broadcast_to([128, N, M_WIDE])
                in1_s = ppt[h][:, 0:M_WIDE].rearrange("p (o t) -> p o t", o=1).broadcast_to([128, N, M_WIDE])
                ts_w = trash.tile([128, N * M_WIDE], BF16, tag="trWS", bufs=4)
                nc.vector.scalar_tensor_tensor(
                    ts_w[:].rearrange("p (l t) -> p l t", l=N, t=M_WIDE),
                    in0_s, -MARGIN, in1_s, op0=ALU.add, op1=ALU.min,
                    accum_out=acc[:, base + WSUM_COL: base + WSUM_COL + 1],
                )
                ppd = small.tile([128, M_WIDE], F32, tag="ppd")
                nc.vector.tensor_scalar(
                    ppd[:], ppt[h][:, 0:M_WIDE], 1.0, None,
                    op0=ALU.mult, op1=ALU.add,
                    accum_out=acc[:, base + PPSUM_COL: base + PPSUM_COL + 1],
                )
                # narrow ACT relu-sums for the remaining slots
                for t in range(M_WIDE, K):
                    a_sum = acc[:, base + (t - M_WIDE): base + (t - M_WIDE) + 1]
                    t1 = trash.tile([128, N], BF16, tag="trA")
                    nc.scalar.activation(
                        t1[:], dm[:], ACTF.Relu,
                        bias=pm[h][:, t: t + 1], scale=-1.0,
                        accum_out=a_sum,
                    )

            j = p * ACC_W * HALVES
            nc.tensor.matmul(
                fin_ps[0:1, j: j + ACC_W * HALVES], ones_col[:], acc[:],
                start=True, stop=True,
            )

        # ---- finalize ----
        fin = small.tile([1, ACC_W * HALVES * PPC], F32, tag="fin")
        nc.vector.tensor_copy(fin[:], fin_ps[:])
        nblk = PPC * HALVES
        s_ph = small.tile([1, nblk], F32, tag="s_ph")
        nc.vector.tensor_reduce(
            s_ph[:],
            fin[:].rearrange("o (j w) -> o j w", j=nblk, w=ACC_W)[:, :, 0:SA],
            axis=mybir.AxisListType.X, op=ALU.add,
        )
        # wide-sum (min form): S += N * ppsum - widemin
        wadj = small.tile([1, nblk], F32, tag="wadj")
        nc.vector.tensor_scalar(
            wadj[:],
            fin[:].rearrange("o (j w) -> o j w", j=nblk, w=ACC_W)[:, :, PPSUM_COL:PPSUM_COL + 1],
            float(N), None, op0=ALU.mult,
        )
        nc.vector.tensor_tensor(s_ph[:], s_ph[:], wadj[:], op=ALU.add)
        nc.vector.tensor_tensor(
            s_ph[:], s_ph[:],
            fin[:].rearrange("o (j w) -> o j w", j=nblk, w=ACC_W)[:, :, WSUM_COL:WSUM_COL + 1],
            op=ALU.subtract,
        )
        n_ph = small.tile([1, nblk], F32, tag="n_ph")
        nc.vector.tensor_copy(
            n_ph[:],
            fin[:].rearrange("o (j w) -> o j w", j=nblk, w=ACC_W)[:, :, CNT_COL:CNT_COL + 1],
        )
        s_p = small.tile([1, PPC], F32, tag="s_p")
        nc.vector.tensor_reduce(
            s_p[:], s_ph[:].rearrange("o (p h) -> o p h", p=PPC, h=HALVES),
            axis=mybir.AxisListType.X, op=ALU.add,
        )
        n_p = small.tile([1, PPC], F32, tag="n_p")
        nc.vector.tensor_reduce(
            n_p[:], n_ph[:].rearrange("o (p h) -> o p h", p=PPC, h=HALVES),
            axis=mybir.AxisListType.X, op=ALU.add,
        )
        both = small.tile([1, 2 * PPC], F32, tag="both")
        nc.gpsimd.tensor_copy(both[:, 0:PPC], s_p[:])
        nc.gpsimd.tensor_copy(both[:, PPC: 2 * PPC], n_p[:])
        nc.sync.dma_start(sn_out[:], both[:])

    if do_compile:
        nc.compile()
    return nc


_CST_CACHE = None


def _const_blob():
    global _CST_CACHE
    if _CST_CACHE is None:
        import ml_dtypes
        cst = np.zeros((128, 640), np.float32)
        ct = (np.arange(N)[None, :] // K == np.arange(NCLS)[:, None]).astype(np.float32)
        # lhsT_st: row 0 = -1/2, rows 1..32 = -L/2 * CT
        cst[0, 0:N] = -0.5
        cst[1: NCLS + 1, 0:N] = -LBIG / 2 * ct
        # rhs_st: row 0 dynamic (sqrow), rows 1..32 = CT
        cst[1: NCLS + 1, N: 2 * N] = ct
        # ident
        cst[:, 2 * N: 2 * N + 128] = np.eye(128, dtype=np.float32)
        _CST_CACHE = cst.astype(ml_dtypes.bfloat16)
    return _CST_CACHE


_NC_CACHE = None


def _get_nc():
    global _NC_CACHE
    if _NC_CACHE is None:
        _NC_CACHE = build_kernel()
    return _NC_CACHE


def kernel(x: np.ndarray, y: np.ndarray) -> np.ndarray:
    from concourse.bass_utils import run_bass_kernel_spmd

    x = np.asarray(x)
    y = np.asarray(y)
    assert x.shape == (P_TOT, N, C) and y.shape == (P_TOT, N)
    expect = np.repeat(np.arange(NCLS, dtype=np.int64), K)
    assert np.array_equal(y, np.broadcast_to(expect, (P_TOT, N))), (
        "kernel requires y[p, i] == i // 8"
    )
    nc = _get_nc()
    xs = np.ascontiguousarray(x.reshape(NCORES, PPC * N, C).astype(np.float32))
    cst = _const_blob()
    in_maps = [{"x": xs[i], "cst": cst} for i in range(NCORES)]
    res = run_bass_kernel_spmd(nc, in_maps, list(range(NCORES)))
    out = np.empty((P_TOT,), np.float32)
    for i in range(NCORES):
        sn = res.results[i]["sn"].reshape(2 * PPC)
        s, n = sn[:PPC], sn[PPC:]
        out[i * PPC: (i + 1) * PPC] = np.where(n <= 0, 0.0, s / np.maximum(n, 1.0))
    return out


# revision 36
# speedup vs baseline: 1.0104x; 1.0104x over previous
"""BatchAllTripletLoss Trainium2 kernel (final).

Problem (hardcoded): x (64, 256, 256) f32, y (64, 256) int64 with
y[p, i] = i // 8 (32 classes x 8 members, uniform, identical across parts).
Output: per-part batch-all triplet loss, shape (64,) f32.

Design (8 cores x 8 parts, fully independent):
  - constants (fold lhsT/rhs templates, identity) DMA'd from a host-
    built DRAM blob on the ACT queue (no Pool affine_selects at boot)
  - x loaded f32 (sync DMA), cast to bf16 on GpSimd (Pool)
  - squared norms on ACT (Square + accum); x^T via PE transposes with
    the sqrow strip sharing the transpose PSUM bank
  - one PSUM accumulation per part [128,512]: gram (2 matmuls/half) +
    stacked [33,*] fold (row0 = -sqrow/2, rows 1-32 = -L/2 * CT)
  - D' = sqrt in bf16 on ACT; pos distances from the SAME psum:
    ps-space min over the anchor-half's 16-class block, clamped via
    sposc2 = min(2*spos, sq-L), pp = sqrt(-sposc2+(sq-L))
  - epilogue per (part, half): ALL 8 counts in ONE wide DVE
    scalar_tensor_tensor ((D - m) is_lt pp_broadcast, [128, 2048],
    in-op accumulated, t-major layout); relu-sums: M_WIDE slots in one
    wide DVE op using the exact min identity
    relu(pp+m-D) = pp - min(D-m, pp)  =>  S = N*sum(pp) - sum(min),
    the rest as narrow ACT Relu ops (bias=pm, scale=-1, accum)
  - per-core output: (S_p, N_p) pairs; host does the final division.
"""

import numpy as np
from contextlib import ExitStack

import concourse.bass as bass
import concourse.bacc as bacc_mod
import concourse.mybir as mybir
import concourse.tile as tile

F32 = mybir.dt.float32
BF16 = mybir.dt.bfloat16
ALU = mybir.AluOpType
ACTF = mybir.ActivationFunctionType

P_TOT, N, C = 64, 256, 256
K, NCLS = 8, 32
MARGIN = 0.2
NCORES = 8
PPC = P_TOT // NCORES
HALVES = 2
LBIG = float(2 << 19)
# slots 0..M_WIDE-1 of the relu-sums ride ONE wide DVE op (broadcast
# APs, min-form corrected by N*sum(pp)); the rest are narrow ACT Relu
# ops. All counts ride ONE wide DVE op per (part, half).
M_WIDE = 3
SA = K - M_WIDE          # narrow ACT relu-sum slots -> acc cols 0..SA-1
WSUM_COL = SA            # wide-sum accumulator col
PPSUM_COL = SA + 1       # sum of pp over wide slots (for min-form correction)
CNT_COL = SA + 2         # wide-count accumulator col
ACC_W = SA + 3


def build_kernel(do_compile=True, reps=1):
    nc = bacc_mod.Bacc()
    x_in = nc.declare_dram_parameter("x", [PPC * N, C], F32, isOutput=False)
    cst_in = nc.declare_dram_parameter("cst", [128, 640], BF16, isOutput=False)
    sn_out = nc.declare_dram_parameter("sn", [1, 2 * PPC], F32, isOutput=True)

    with tile.TileContext(nc) as tc, ExitStack() as ctx:
        consts = ctx.enter_context(tc.tile_pool(name="consts", bufs=1))
        xpool = ctx.enter_context(tc.tile_pool(name="xpool", bufs=2))
        xtpool = ctx.enter_context(tc.tile_pool(name="xtpool", bufs=2))
        dpool = ctx.enter_context(tc.tile_pool(name="dpool", bufs=4))
        small = ctx.enter_context(tc.tile_pool(name="small", bufs=6))
        trash = ctx.enter_context(tc.tile_pool(name="trash", bufs=8))
        accp = ctx.enter_context(tc.tile_pool(name="accp", bufs=3))
        psum = ctx.enter_context(tc.tile_pool(name="psum", bufs=2, space="PSUM"))
        psmall = ctx.enter_context(tc.tile_pool(name="psmall", bufs=1, space="PSUM"))

        # ---- one-time constants, DMA'd from DRAM on the ACT queue ----
        lhsT_st = consts.tile([NCLS + 1, N], BF16, tag="lhsT_st")
        nc.scalar.dma_start(lhsT_st[:], cst_in[0: NCLS + 1, 0:N])
        rhs_st = consts.tile([NCLS + 1, N], BF16, tag="rhs_st")
        nc.scalar.dma_start(rhs_st[:], cst_in[0: NCLS + 1, N: 2 * N])
        ident = consts.tile([128, 128], BF16, tag="ident")
        nc.scalar.dma_start(ident[:], cst_in[:, 2 * N: 2 * N + 128])
        ones_col = consts.tile([128, 1], F32, tag="ones_col")
        nc.gpsimd.memset(ones_col[:], 1.0)

        fin_ps = psmall.tile([1, ACC_W * HALVES * PPC], F32, tag="fin_ps")

        for p in [pp for _ in range(reps) for pp in range(PPC)]:
            # ---- load f32, cast to bf16 on Pool ----
            xf = [xpool.tile([128, C], F32, tag="xf", name="xf", bufs=6) for _ in range(HALVES)]
            for h in range(HALVES):
                nc.sync.dma_start(xf[h][:], x_in[p * N + 128 * h: p * N + 128 * (h + 1), :])
            xb = [xpool.tile([128, C], BF16, tag="xb", name="xb") for _ in range(HALVES)]
            for h in range(HALVES):
                nc.gpsimd.tensor_copy(xb[h][:], xf[h][:])

            # ---- squared norms ----
            sqcol2 = small.tile([128, HALVES], F32, tag="sqcol2")
            for h in range(HALVES):
                st = trash.tile([128, C], BF16, tag="trash_sq")
                nc.scalar.activation(
                    st[:], xb[h][:], ACTF.Square,
                    accum_out=sqcol2[:, h: h + 1],
                )
            scb = small.tile([128, HALVES], BF16, tag="scb")
            nc.gpsimd.tensor_copy(scb[:], sqcol2[:])

            # ---- transposes via PE ----
            xtps = psum.tile([128, 2 * N + 2 * 128], BF16, tag="xtps", name="xtps", bufs=2)
            for cchunk in range(2):
                for h in range(HALVES):
                    nc.tensor.transpose(
                        xtps[:, 256 * cchunk + 128 * h: 256 * cchunk + 128 * (h + 1)],
                        xb[h][:, 128 * cchunk: 128 * (cchunk + 1)],
                        ident[:],
                    )
            for h in range(HALVES):
                nc.tensor.transpose(
                    xtps[0:1, 2 * N + 128 * h: 2 * N + 128 * (h + 1)],
                    scb[:, h: h + 1], ident[:],
                )
            xtb_all = xtpool.tile([128, 2 * N], BF16, tag="xtb", name="xtb")
            nc.vector.tensor_copy(xtb_all[:], xtps[:, 0: 2 * N])
            xtb = [xtb_all[:, 0:N], xtb_all[:, N: 2 * N]]
            nc.vector.tensor_copy(rhs_st[0:1, :], xtps[0:1, 2 * N: 2 * N + 2 * 128])

            # ---- psum: gram + stacked fold ----
            ps = psum.tile([128, 2 * N], F32, tag="ps", bufs=3)
            for h in range(HALVES):
                psh = ps[:, N * h: N * (h + 1)]
                nc.tensor.matmul(
                    psh, xtb[0][:, 128 * h: 128 * (h + 1)], xtb[0][:],
                    start=True, stop=False,
                )
                nc.tensor.matmul(
                    psh, xtb[1][:, 128 * h: 128 * (h + 1)], xtb[1][:],
                    start=False, stop=False,
                )
                nc.tensor.matmul(
                    psh, lhsT_st[:, 128 * h: 128 * (h + 1)], rhs_st[:],
                    start=False, stop=True,
                )

            sqml = small.tile([128, HALVES], F32, tag="sqml")
            nc.vector.tensor_scalar(sqml[:], sqcol2[:], -LBIG, None, op0=ALU.add)

            acc = accp.tile([128, ACC_W * HALVES], F32, tag="acc", name="acc")
            dmat = []
            pm = []
            ppt = []
            for h in range(HALVES):
                psh = ps[:, N * h: N * (h + 1)]
                dm = dpool.tile([128, N], BF16, tag="dmat")
                nc.scalar.activation(
                    dm[:], psh, ACTF.Sqrt, bias=sqcol2[:, h: h + 1], scale=-2.0,
                )
                dmat.append(dm)

                spos = small.tile([128, K], F32, tag="spos")
                nc.vector.tensor_reduce(
                    spos[:],
                    ps[:, 384 * h: 384 * h + 128].rearrange(
                        "p (cc t) -> p t cc", cc=NCLS // 2, t=K
                    ),
                    axis=mybir.AxisListType.X, op=ALU.min,
                )
                sposc2 = small.tile([128, K], F32, tag="sposc2")
                nc.vector.tensor_scalar(
                    sposc2[:], spos[:], 2.0, sqml[:, h: h + 1],
                    op0=ALU.mult, op1=ALU.min,
                )
                pp = small.tile([128, K], F32, tag="pp")
                nc.scalar.activation(
                    pp[:], sposc2[:], ACTF.Sqrt, bias=sqml[:, h: h + 1], scale=-1.0,
                )
                ppt.append(pp)
                pmh = small.tile([128, K], F32, tag="pmh")
                nc.vector.tensor_scalar(pmh[:], pp[:], MARGIN, None, op0=ALU.add)
                pm.append(pmh)

            # ---- epilogue slots ----
            for h in range(HALVES):
                dm = dmat[h]
                base = ACC_W * h
                # wide count: all 8 slots in one broadcast-AP STT
                in0_w = dm[:, :].rearrange("p (o l) -> p o l", o=1).broadcast_to([128, K, N])
                in1_w = ppt[h][:, :].rearrange("p (t o) -> p t o", o=1).broadcast_to([128, K, N])
                tc_w = trash.tile([128, N * K], BF16, tag="trWC", bufs=4)
                nc.vector.scalar_tensor_tensor(
                    tc_w[:].rearrange("p (t l) -> p t l", t=K, l=N),
                    in0_w, -MARGIN, in1_w, op0=ALU.add, op1=ALU.is_lt,
                    accum_out=acc[:, base + CNT_COL: base + CNT_COL + 1],
                )
                # wide sum over slots 0..M_WIDE-1: accum = sum max(pm_t, D)
                # min form: relu(pp+m-D) = pp - min(D-m, pp), exactly; so
                # wide relu-sum = N*sum(pp[0:M]) - sum min(D-m, pp)
                in0_s = dm[:, :].rearrange("p (l o) -> p l o", o=1).broadcast_to([128, N, M_WIDE])
                in1_s = ppt[h][:, 0:M_WIDE].rearrange("p (o t) -> p o t", o=1).broadcast_to([128, N, M_WIDE])
                ts_w = trash.tile([128, N * M_WIDE], BF16, tag="trWS", bufs=4)
                nc.vector.scalar_tensor_tensor(
                    ts_w[:].rearrange("p (l t) -> p l t", l=N, t=M_WIDE),
                    in0_s, -MARGIN, in1_s, op0=ALU.add, op1=ALU.min,
                    accum_out=acc[:, base + WSUM_COL: base + WSUM_COL + 1],
                )
                ppd = small.tile([128, M_WIDE], F32, tag="ppd")
                nc.vector.tensor_scalar(
                    ppd[:], ppt[h][:, 0:M_WIDE], 1.0, None,
                    op0=ALU.mult, op1=ALU.add,
                    accum_out=acc[:, base + PPSUM_COL: base + PPSUM_COL + 1],
                )
                # narrow ACT relu-sums for the remaining slots
                for t in range(M_WIDE, K):
                    a_sum = acc[:, base + (t - M_WIDE): base + (t - M_WIDE) + 1]
                    t1 = trash.tile([128, N], BF16, tag="trA")
                    nc.scalar.activation(
                        t1[:], dm[:], ACTF.Relu,
                        bias=pm[h][:, t: t + 1], scale=-1.0,
                        accum_out=a_sum,
                    )

            j = p * ACC_W * HALVES
            nc.tensor.matmul(
                fin_ps[0:1, j: j + ACC_W * HALVES], ones_col[:], acc[:],
                start=True, stop=True,
            )

        # ---- finalize ----
        fin = small.tile([1, ACC_W * HALVES * PPC], F32, tag="fin")
        nc.vector.tensor_copy(fin[:], fin_ps[:])
        nblk = PPC * HALVES
        s_ph = small.tile([1, nblk], F32, tag="s_ph")
        nc.vector.tensor_reduce(
            s_ph[:],
            fin[:].rearrange("o (j w) -> o j w", j=nblk, w=ACC_W)[:, :, 0:SA],
            axis=mybir.AxisListType.X, op=ALU.add,
        )
        # wide-sum (min form): S += N * ppsum - widemin
        wadj = small.tile([1, nblk], F32, tag="wadj")
        nc.vector.tensor_scalar(
            wadj[:],
            fin[:].rearrange("o (j w) -> o j w", j=nblk, w=ACC_W)[:, :, PPSUM_COL:PPSUM_COL + 1],
            float(N), None, op0=ALU.mult,
        )
        nc.vector.tensor_tensor(s_ph[:], s_ph[:], wadj[:], op=ALU.add)
        nc.vector.tensor_tensor(
            s_ph[:], s_ph[:],
            fin[:].rearrange("o (j w) -> o j w", j=nblk, w=ACC_W)[:, :, WSUM_COL:WSUM_COL + 1],
            op=ALU.subtract,
        )
        n_ph = small.tile([1, nblk], F32, tag="n_ph")
        nc.vector.tensor_copy(
            n_ph[:],
            fin[:].rearrange("o (j w) -> o j w", j=nblk, w=ACC_W)[:, :, CNT_COL:CNT_COL + 1],
        )
        s_p = small.tile([1, PPC], F32, tag="s_p")
        nc.vector.tensor_reduce(
            s_p[:], s_ph[:].rearrange("o (p h) -> o p h", p=PPC, h=HALVES),
            axis=mybir.AxisListType.X, op=ALU.add,
        )
        n_p = small.tile([1, PPC], F32, tag="n_p")
        nc.vector.tensor_reduce(
            n_p[:], n_ph[:].rearrange("o (p h) -> o p h", p=PPC, h=HALVES),
            axis=mybir.AxisListType.X, op=ALU.add,
        )
        both = small.tile([1, 2 * PPC], F32, tag="both")
        nc.gpsimd.tensor_copy(both[:, 0:PPC], s_p[:])
        nc.gpsimd.tensor_copy(both[:, PPC: 2 * PPC], n_p[:])
        nc.sync.dma_start(sn_out[:], both[:])

    if do_compile:
        nc.compile()
    return nc


_CST_CACHE = None


def _const_blob():
    global _CST_CACHE
    if _CST_CACHE is None:
        import ml_dtypes
        cst = np.zeros((128, 640), np.float32)
        ct = (np.arange(N)[None, :] // K == np.arange(NCLS)[:, None]).astype(np.float32)
        # lhsT_st: row 0 = -1/2, rows 1..32 = -L/2 * CT
        cst[0, 0:N] = -0.5
        cst[1: NCLS + 1, 0:N] = -LBIG / 2 * ct
        # rhs_st: row 0 dynamic (sqrow), rows 1..32 = CT
        cst[1: NCLS + 1, N: 2 * N] = ct
        # ident
        cst[:, 2 * N: 2 * N + 128] = np.eye(128, dtype=np.float32)
        _CST_CACHE = cst.astype(ml_dtypes.bfloat16)
    return _CST_CACHE


_NC_CACHE = None


def _get_nc():
    global _NC_CACHE
    if _NC_CACHE is None:
        _NC_CACHE = build_kernel()
    return _NC_CACHE


def kernel(x: np.ndarray, y: np.ndarray) -> np.ndarray:
    from concourse.bass_utils import run_bass_kernel_spmd

    x = np.asarray(x)
    y = np.asarray(y)
    assert x.shape == (P_TOT, N, C) and y.shape == (P_TOT, N)
    expect = np.repeat(np.arange(NCLS, dtype=np.int64), K)
    assert np.array_equal(y, np.broadcast_to(expect, (P_TOT, N))), (
        "kernel requires y[p, i] == i // 8"
    )
    nc = _get_nc()
    xs = np.ascontiguousarray(x.reshape(NCORES, PPC * N, C).astype(np.float32))
    cst = _const_blob()
    in_maps = [{"x": xs[i], "cst": cst} for i in range(NCORES)]
    res = run_bass_kernel_spmd(nc, in_maps, list(range(NCORES)))
    out = np.empty((P_TOT,), np.float32)
    for i in range(NCORES):
        sn = res.results[i]["sn"].reshape(2 * PPC)
        s, n = sn[:PPC], sn[PPC:]
        out[i * PPC: (i + 1) * PPC] = np.where(n <= 0, 0.0, s / np.maximum(n, 1.0))
    return out
